# revision 2
# baseline (speedup 1.0000x reference)
"""Trainium2 Bass kernel for nn_Attention_5420248728069.

Computes, for full inputs (sharded data-parallel over 8 NeuronCores on v_code
rows; obs_code and weights replicated; no collectives):

    v_value   = v @ Wv.T ; obs_value = obs @ Wv.T
    v_query   = v @ Wq.T ; v_key = v @ Wk.T ; obs_key = obs @ Wk.T
    S         = v_query @ obs_key.T            # cross attention [N, M]
    s_self    = rowsum(v_query * v_key)        # [N]
    w         = softmax(concat([s_self, S]) / sqrt(E))
    out       = LayerNorm(w0 * v_value + w[:,1:] @ obs_value + v) * gamma + beta

Algebraic refactoring used by the kernel (exact in f32):
    A   = (Wq.T @ Wk) / TEMP                   # [E, E], computed once
    S/T = (v @ A) @ obs.T ;  s_self/T = rowsum((v@A) * v)
    y   = (w0 * v + expS @ obs) @ Wv.T / Z + v # unnormalized-softmax form

Compute dtype: bf16 on TensorE with f32 PSUM accumulation (verified
rel-l2-err ~5e-4 vs the f32 reference).
"""

import numpy as np

N_GLOBAL = 8192
M = 4096
E = 512
CORES = 8
NLOC = N_GLOBAL // CORES  # 1024
TEMPERATURE = 22.627416997969522  # sqrt(E)
EPS = 1e-6
P = 128

_CACHED_NC = None


def _build():
    from contextlib import ExitStack

    import concourse.bass as bass
    import concourse.tile as tile
    from concourse import bacc, mybir
    from concourse.masks import make_identity

    f32 = mybir.dt.float32
    bf16 = mybir.dt.bfloat16
    AF = mybir.ActivationFunctionType
    ALU = mybir.AluOpType

    nc = bacc.Bacc("TRN2", target_bir_lowering=False, debug=False)

    v_d = nc.dram_tensor("v_code", [NLOC, E], f32, kind="ExternalInput")
    obs_d = nc.dram_tensor("obs_code", [M, E], f32, kind="ExternalInput")
    wq_d = nc.dram_tensor("Wq", [E, E], f32, kind="ExternalInput")
    wk_d = nc.dram_tensor("Wk", [E, E], f32, kind="ExternalInput")
    wv_d = nc.dram_tensor("Wv", [E, E], f32, kind="ExternalInput")
    gamma_d = nc.dram_tensor("gamma", [E], f32, kind="ExternalInput")
    beta_d = nc.dram_tensor("beta", [E], f32, kind="ExternalInput")
    out_d = nc.dram_tensor("out", [NLOC, E], f32, kind="ExternalOutput")

    def bcast_ap(ap_1row, parts=P):
        # replicate a [1, F] (or [F]) DRAM AP across `parts` partitions
        dims = [list(d) for d in ap_1row.ap]
        if len(dims) > 1 and dims[0][1] == 1:
            dims = dims[1:]
        return bass.AP(
            tensor=ap_1row.tensor, offset=ap_1row.offset, ap=[[0, parts]] + dims
        )

    with tile.TileContext(nc) as tc, ExitStack() as ctx:
        const = ctx.enter_context(tc.tile_pool(name="const", bufs=1))
        persist = ctx.enter_context(tc.tile_pool(name="persist", bufs=1))
        dram = ctx.enter_context(tc.tile_pool(name="dram", bufs=1, space="DRAM"))
        expp = ctx.enter_context(tc.tile_pool(name="expp", bufs=3))
        epi = ctx.enter_context(tc.tile_pool(name="epi", bufs=2))

        # ---- persistent SBUF tensors
        v_f32 = persist.tile([P, 8, E], f32, tag="v_f32")
        vT = persist.tile([P, 4, NLOC], bf16, tag="vT")
        vAT = persist.tile([P, 4, NLOC], bf16, tag="vAT")
        A_sb = persist.tile([P, 4, E], bf16, tag="A")
        WvT = persist.tile([P, 4, E], bf16, tag="WvT")
        obs_bf = persist.tile([P, 32, E], bf16, tag="obs_bf")
        obsT = persist.tile([P, 4, M], bf16, tag="obsT")
        uT = persist.tile([P, 4, NLOC], bf16, tag="uT")
        w0 = persist.tile([P, 8], f32, tag="w0")
        w0_bc = persist.tile([P, NLOC], f32, tag="w0_bc")
        ztok = persist.tile([P, 8], f32, tag="ztok")
        recipZ = persist.tile([P, 8], f32, tag="recipZ")

        gamma_b = const.tile([P, E], f32, tag="gamma")
        beta_b = const.tile([P, E], f32, tag="beta")
        identity = const.tile([P, P], bf16, tag="ident")
        ones_bf = const.tile([P, 1], bf16, tag="ones")
        eps_t = const.tile([P, 1], f32, tag="eps")

        make_identity(nc, identity)
        nc.vector.memset(ones_bf, 1.0)
        nc.vector.memset(eps_t, EPS)
        nc.gpsimd.dma_start(out=gamma_b, in_=bcast_ap(gamma_d.ap()))
        nc.gpsimd.dma_start(out=beta_b, in_=bcast_ap(beta_d.ap()))

        scr_v = dram.tile([NLOC, E], bf16, tag="scr_v")
        scr_obs = dram.tile([M, E], bf16, tag="scr_obs")
        scr_z = dram.tile([1, NLOC], f32, tag="scr_z")
        scr_w0 = dram.tile([1, NLOC], f32, tag="scr_w0")

        out_r = out_d.ap().rearrange("(c p) e -> p c e", p=P)

        with ExitStack() as sctx:
            stage = sctx.enter_context(tc.tile_pool(name="stage", bufs=2))
            setup = sctx.enter_context(tc.tile_pool(name="setup", bufs=1))
            pstage = sctx.enter_context(
                tc.tile_pool(name="pstage", bufs=2, space="PSUM")
            )

            # ---- obs pipeline: load f32 -> cast bf16 -> scratch -> obsT
            obs_r = obs_d.ap().rearrange("(c p) e -> p c e", p=P)
            scr_obs_r = scr_obs.rearrange("(c p) e -> p c e", p=P)
            for lc in range(8):
                of = stage.tile([P, 4, E], f32, tag="obs_f")
                nc.sync.dma_start(of, obs_r[:, lc * 4 : (lc + 1) * 4, :])
                nc.vector.tensor_copy(obs_bf[:, lc * 4 : (lc + 1) * 4, :], of)
                nc.sync.dma_start(
                    scr_obs_r[:, lc * 4 : (lc + 1) * 4, :],
                    obs_bf[:, lc * 4 : (lc + 1) * 4, :],
                )
                for ec in range(4):
                    nc.sync.dma_start_transpose(
                        obsT[:, ec, lc * 512 : (lc + 1) * 512],
                        scr_obs[lc * 512 : (lc + 1) * 512, ec * P : (ec + 1) * P],
                    )

            # ---- weights: A = (Wq.T @ Wk) / TEMP
            wq_f = stage.tile([P, 4, E], f32, tag="w_f")
            nc.sync.dma_start(wq_f, wq_d.ap().rearrange("(c p) e -> p c e", p=P))
            wq_b = setup.tile([P, 4, E], bf16, tag="wq_b")
            nc.vector.tensor_copy(wq_b, wq_f)
            wk_f = stage.tile([P, 4, E], f32, tag="w_f")
            nc.sync.dma_start(wk_f, wk_d.ap().rearrange("(c p) e -> p c e", p=P))
            wk_b = setup.tile([P, 4, E], bf16, tag="wk_b")
            nc.vector.tensor_copy(wk_b, wk_f)
            for ic in range(4):
                psA = pstage.tile([P, E], f32, tag="psA")
                for kc in range(4):
                    nc.tensor.matmul(
                        psA,
                        lhsT=wq_b[:, kc, ic * P : (ic + 1) * P],
                        rhs=wk_b[:, kc, :],
                        start=(kc == 0),
                        stop=(kc == 3),
                    )
                nc.scalar.mul(A_sb[:, ic, :], psA, 1.0 / TEMPERATURE)

            # ---- WvT (transpose Wv via TensorE)
            wv_f = stage.tile([P, 4, E], f32, tag="w_f")
            nc.sync.dma_start(wv_f, wv_d.ap().rearrange("(c p) e -> p c e", p=P))
            wv_b = setup.tile([P, 4, E], bf16, tag="wv_b")
            nc.vector.tensor_copy(wv_b, wv_f)
            for ic in range(4):  # e_out chunk of Wv
                for jc in range(4):  # e_in slice
                    pst = pstage.tile([P, P], bf16, tag="pst")
                    nc.tensor.transpose(
                        pst, wv_b[:, ic, jc * P : (jc + 1) * P], identity
                    )
                    nc.vector.tensor_copy(WvT[:, jc, ic * P : (ic + 1) * P], pst)

            # ---- v: load, cast, vT via DMA transpose
            nc.sync.dma_start(v_f32, v_d.ap().rearrange("(c p) e -> p c e", p=P))
            v_bf = setup.tile([P, 8, E], bf16, tag="v_bf")
            nc.vector.tensor_copy(v_bf, v_f32)
            nc.sync.dma_start(scr_v.rearrange("(c p) e -> p c e", p=P), v_bf)
            for ec in range(4):
                nc.sync.dma_start_transpose(
                    vT[:, ec, :], scr_v[:, ec * P : (ec + 1) * P]
                )

            # ---- vAT = (v @ A).T   [e2, n]
            for e2 in range(4):
                for nb in range(2):
                    psv = pstage.tile([P, 512], f32, tag="psv")
                    for e1 in range(4):
                        nc.tensor.matmul(
                            psv,
                            lhsT=A_sb[:, e1, e2 * P : (e2 + 1) * P],
                            rhs=vT[:, e1, nb * 512 : (nb + 1) * 512],
                            start=(e1 == 0),
                            stop=(e1 == 3),
                        )
                    nc.vector.tensor_copy(vAT[:, e2, nb * 512 : (nb + 1) * 512], psv)

            # ---- self score (token-major [n,1] per chunk) and w0 = exp(.)
            prod = setup.tile([P, 4, NLOC], bf16, tag="prod")
            for ec in range(4):
                nc.vector.tensor_mul(prod[:, ec, :], vAT[:, ec, :], vT[:, ec, :])
            ps_sf = pstage.tile([P, 8], f32, tag="ps_sf")
            for nk in range(8):
                for ec in range(4):
                    nc.tensor.matmul(
                        ps_sf[:, nk : nk + 1],
                        lhsT=prod[:, ec, nk * P : (nk + 1) * P],
                        rhs=ones_bf,
                        start=(ec == 0),
                        stop=(ec == 3),
                    )
            nc.scalar.activation(w0, ps_sf, AF.Exp)
            # w0 row-major broadcast [P, NLOC] via DRAM roundtrip
            nc.sync.dma_start(scr_w0.rearrange("o (a p) -> (o p) a", p=P), w0)
            nc.gpsimd.dma_start(w0_bc, bcast_ap(scr_w0[:]))

        # ---- main loop + fused epilogue
        with ExitStack() as mctx:
            ps_ut_pool = mctx.enter_context(
                tc.tile_pool(name="ps_ut", bufs=1, space="PSUM")
            )
            ps_s_pool = mctx.enter_context(
                tc.tile_pool(name="ps_s", bufs=2, space="PSUM")
            )
            ps_z_pool = mctx.enter_context(
                tc.tile_pool(name="ps_z", bufs=1, space="PSUM")
            )
            ps_y_pool = mctx.enter_context(
                tc.tile_pool(name="ps_y", bufs=1, space="PSUM")
            )
            for nb in range(2):
                nsl = slice(nb * 512, (nb + 1) * 512)
                ps_uT = ps_ut_pool.tile([P, 4, 512], f32, tag="uT")
                ps_z = ps_z_pool.tile([1, 512], f32, tag="z")
                for mc in range(32):
                    ps_s = ps_s_pool.tile([P, 512], f32, tag="s")
                    for ec in range(4):
                        nc.tensor.matmul(
                            ps_s,
                            lhsT=obsT[:, ec, mc * P : (mc + 1) * P],
                            rhs=vAT[:, ec, nsl],
                            start=(ec == 0),
                            stop=(ec == 3),
                        )
                    ex = expp.tile([P, 512], bf16, tag="ex")
                    nc.scalar.activation(ex, ps_s, AF.Exp)
                    nc.tensor.matmul(
                        ps_z, lhsT=ones_bf, rhs=ex, start=(mc == 0), stop=(mc == 31)
                    )
                    for es in range(4):
                        nc.tensor.matmul(
                            ps_uT[:, es, :],
                            lhsT=obs_bf[:, mc, es * P : (es + 1) * P],
                            rhs=ex,
                            start=(mc == 0),
                            stop=(mc == 31),
                        )
                # drain uT (+ fold in w0 * v term) and Z for this n-block
                for ec in range(4):
                    tmp = epi.tile([P, 512], bf16, tag="tmpw")
                    nc.vector.tensor_mul(tmp, vT[:, ec, nsl], w0_bc[:, nsl])
                    nc.vector.tensor_add(uT[:, ec, nsl], tmp, ps_uT[:, ec, :])
                zrow = epi.tile([1, 512], f32, tag="zrow")
                nc.scalar.copy(zrow, ps_z)
                nc.sync.dma_start(scr_z[:, nsl], zrow)
                nc.sync.dma_start(
                    ztok[:, nb * 4 : (nb + 1) * 4],
                    scr_z[:, nsl].rearrange("o (a p) -> (o p) a", p=P),
                )
                c4 = slice(nb * 4, (nb + 1) * 4)
                nc.vector.tensor_add(ztok[:, c4], ztok[:, c4], w0[:, c4])
                nc.vector.reciprocal(recipZ[:, c4], ztok[:, c4])

                # epilogue for this n-block's 4 token chunks
                for nk in range(nb * 4, (nb + 1) * 4):
                    ps_y = ps_y_pool.tile([P, E], f32, tag="y")
                    for ec in range(4):
                        nc.tensor.matmul(
                            ps_y,
                            lhsT=uT[:, ec, nk * P : (nk + 1) * P],
                            rhs=WvT[:, ec, :],
                            start=(ec == 0),
                            stop=(ec == 3),
                        )
                    y2 = epi.tile([P, E], f32, tag="y2")
                    nc.vector.scalar_tensor_tensor(
                        y2,
                        in0=ps_y,
                        scalar=recipZ[:, nk : nk + 1],
                        in1=v_f32[:, nk, :],
                        op0=ALU.mult,
                        op1=ALU.add,
                    )
                    stats = epi.tile([P, 6], f32, tag="stats")
                    nc.vector.bn_stats(stats, y2)
                    mv = epi.tile([P, 2], f32, tag="mv")
                    nc.vector.bn_aggr(mv, stats)
                    std = epi.tile([P, 1], f32, tag="std")
                    nc.scalar.activation(std, mv[:, 1:2], AF.Sqrt, bias=eps_t)
                    rstd = epi.tile([P, 1], f32, tag="rstd")
                    nc.vector.reciprocal(rstd, std)
                    y3 = epi.tile([P, E], f32, tag="y3")
                    nc.vector.tensor_scalar(
                        y3,
                        in0=y2,
                        scalar1=mv[:, 0:1],
                        scalar2=rstd,
                        op0=ALU.subtract,
                        op1=ALU.mult,
                    )
                    y4 = epi.tile([P, E], f32, tag="y4")
                    nc.vector.tensor_mul(y4, y3, gamma_b)
                    y5 = epi.tile([P, E], f32, tag="y5")
                    nc.vector.tensor_add(y5, y4, beta_b)
                    nc.sync.dma_start(out_r[:, nk, :], y5)

    nc.compile()
    return nc


def _get_nc():
    global _CACHED_NC
    if _CACHED_NC is None:
        _CACHED_NC = _build()
    return _CACHED_NC


def _in_maps(v_code, obs_code, Wq, Wk, Wv, gamma, beta):
    def f(x):
        return np.ascontiguousarray(np.asarray(x), dtype=np.float32)

    shared = {
        "obs_code": f(obs_code),
        "Wq": f(Wq),
        "Wk": f(Wk),
        "Wv": f(Wv),
        "gamma": f(gamma),
        "beta": f(beta),
    }
    return [
        {"v_code": f(v_code[c * NLOC : (c + 1) * NLOC]), **shared}
        for c in range(CORES)
    ]


def run(trace=False, **inputs):
    from concourse.bass_utils import run_bass_kernel_spmd

    nc = _get_nc()
    res = run_bass_kernel_spmd(
        nc, _in_maps(**inputs), core_ids=list(range(CORES)), trace=trace
    )
    out = np.concatenate(
        [res.results[c]["out"] for c in range(CORES)], axis=0
    ).astype(np.float32)
    return out, res


def kernel(**inputs) -> np.ndarray:
    out, _ = run(trace=False, **inputs)
    return out


# revision 4
# speedup vs baseline: 1.3111x; 1.3111x over previous
"""Trainium2 Bass kernel for nn_Attention_5420248728069.

Computes, for full inputs (sharded data-parallel over 8 NeuronCores on v_code
rows; obs_code and weights replicated; no collectives):

    v_value   = v @ Wv.T ; obs_value = obs @ Wv.T
    v_query   = v @ Wq.T ; v_key = v @ Wk.T ; obs_key = obs @ Wk.T
    S         = v_query @ obs_key.T            # cross attention [N, M]
    s_self    = rowsum(v_query * v_key)        # [N]
    w         = softmax(concat([s_self, S]) / sqrt(E))
    out       = LayerNorm(w0 * v_value + w[:,1:] @ obs_value + v) * gamma + beta

Algebraic refactoring used by the kernel (exact in f32):
    A   = (Wq.T @ Wk) / TEMP                   # [E, E], computed once
    S.T = ((v @ A) @ obs.T).T ;  s_self = rowsum((v@A) * v)
    y   = (w0 * v + expS @ obs) @ Wv.T / Z + v # unnormalized-softmax form

Compute dtype: bf16 on TensorE with f32 PSUM accumulation (verified
rel-l2-err ~5e-4 vs the f32 reference).

Engine plan: Scalar sequencer dispatches all input loads (HWDGE) + exp;
Sync dispatches obs scratch writes, xbar transposes and output writes;
GpSimd handles small broadcast/roundtrip DMAs; TensorE transposes v/Wv.
"""

import numpy as np

N_GLOBAL = 8192
M = 4096
E = 512
CORES = 8
NLOC = N_GLOBAL // CORES  # 1024
TEMPERATURE = 22.627416997969522  # sqrt(E)
EPS = 1e-6
P = 128

_CACHED_NC = None


def _build():
    from contextlib import ExitStack

    import concourse.bass as bass
    import concourse.tile as tile
    from concourse import bacc, mybir
    from concourse.masks import make_identity

    f32 = mybir.dt.float32
    bf16 = mybir.dt.bfloat16
    AF = mybir.ActivationFunctionType
    ALU = mybir.AluOpType

    nc = bacc.Bacc("TRN2", target_bir_lowering=False, debug=False)

    v_d = nc.dram_tensor("v_code", [NLOC, E], f32, kind="ExternalInput")
    obs_d = nc.dram_tensor("obs_code", [M, E], f32, kind="ExternalInput")
    wq_d = nc.dram_tensor("Wq", [E, E], f32, kind="ExternalInput")
    wk_d = nc.dram_tensor("Wk", [E, E], f32, kind="ExternalInput")
    wv_d = nc.dram_tensor("Wv", [E, E], f32, kind="ExternalInput")
    gamma_d = nc.dram_tensor("gamma", [E], f32, kind="ExternalInput")
    beta_d = nc.dram_tensor("beta", [E], f32, kind="ExternalInput")
    out_d = nc.dram_tensor("out", [NLOC, E], f32, kind="ExternalOutput")

    def bcast_ap(ap_1row, parts=P):
        # replicate a [1, F] (or [F]) DRAM AP across `parts` partitions
        dims = [list(d) for d in ap_1row.ap]
        if len(dims) > 1 and dims[0][1] == 1:
            dims = dims[1:]
        return bass.AP(
            tensor=ap_1row.tensor, offset=ap_1row.offset, ap=[[0, parts]] + dims
        )

    with tile.TileContext(nc) as tc, ExitStack() as ctx:
        const = ctx.enter_context(tc.tile_pool(name="const", bufs=1))
        persist = ctx.enter_context(tc.tile_pool(name="persist", bufs=1))
        dram = ctx.enter_context(tc.tile_pool(name="dram", bufs=1, space="DRAM"))
        expp = ctx.enter_context(tc.tile_pool(name="expp", bufs=3))
        epi = ctx.enter_context(tc.tile_pool(name="epi", bufs=2))

        # ---- persistent SBUF tensors
        v_f32 = persist.tile([P, 8, E], f32, tag="v_f32")
        vT = persist.tile([P, 4, NLOC], bf16, tag="vT")
        vAT = persist.tile([P, 4, NLOC], bf16, tag="vAT")
        A_sb = persist.tile([P, 4, E], bf16, tag="A")
        WvT = persist.tile([P, 4, E], bf16, tag="WvT")
        obs_bf = persist.tile([P, 32, E], bf16, tag="obs_bf")
        obsT = persist.tile([P, 4, M], bf16, tag="obsT")
        uT = persist.tile([P, 4, NLOC], bf16, tag="uT")
        w0 = persist.tile([P, 8], f32, tag="w0")
        w0_bc = persist.tile([P, NLOC], bf16, tag="w0_bc")
        ztok = persist.tile([P, 8], f32, tag="ztok")
        recipZ = persist.tile([P, 8], f32, tag="recipZ")

        gamma_b = const.tile([P, E], f32, tag="gamma")
        beta_b = const.tile([P, E], f32, tag="beta")
        identity = const.tile([P, P], f32, tag="ident")
        ones_bf = const.tile([P, 1], bf16, tag="ones")
        eps_t = const.tile([P, 1], f32, tag="eps")

        make_identity(nc, identity)
        nc.vector.memset(ones_bf, 1.0)
        nc.vector.memset(eps_t, EPS)
        nc.gpsimd.dma_start(out=gamma_b, in_=bcast_ap(gamma_d.ap()))
        nc.gpsimd.dma_start(out=beta_b, in_=bcast_ap(beta_d.ap()))

        scr_obs = dram.tile([M, E], bf16, tag="scr_obs")
        scr_z = dram.tile([1, NLOC], f32, tag="scr_z")
        scr_w0 = dram.tile([1, NLOC], f32, tag="scr_w0")

        out_r = out_d.ap().rearrange("(c p) e -> p c e", p=P)

        with ExitStack() as sctx:
            stage = sctx.enter_context(tc.tile_pool(name="stage", bufs=2))
            setup = sctx.enter_context(tc.tile_pool(name="setup", bufs=2))
            pstage = sctx.enter_context(
                tc.tile_pool(name="pstage", bufs=2, space="PSUM")
            )

            # ---- all input loads, dispatched from the Scalar sequencer
            # (pure prefetch: no dependencies, never blocks dispatch)
            wq_f = stage.tile([P, 4, E], f32, tag="w_f")
            nc.scalar.dma_start(wq_f, wq_d.ap().rearrange("(c p) e -> p c e", p=P))
            wk_f = stage.tile([P, 4, E], f32, tag="w_f")
            nc.scalar.dma_start(wk_f, wk_d.ap().rearrange("(c p) e -> p c e", p=P))
            wv_f = stage.tile([P, 4, E], f32, tag="w_f")
            nc.scalar.dma_start(wv_f, wv_d.ap().rearrange("(c p) e -> p c e", p=P))
            nc.scalar.dma_start(v_f32, v_d.ap().rearrange("(c p) e -> p c e", p=P))
            obs_r = obs_d.ap().rearrange("(c p) e -> p c e", p=P)
            obs_stage = []
            for lc in range(8):
                of = stage.tile([P, 4, E], f32, tag="obs_f")
                nc.scalar.dma_start(of, obs_r[:, lc * 4 : (lc + 1) * 4, :])
                obs_stage.append(of)

            # ---- A = (Wq.T @ Wk) / TEMP
            wq_b = setup.tile([P, 4, E], bf16, tag="wq_b")
            nc.vector.tensor_copy(wq_b, wq_f)
            wk_b = setup.tile([P, 4, E], bf16, tag="wk_b")
            nc.vector.tensor_copy(wk_b, wk_f)
            for ic in range(4):
                psA = pstage.tile([P, E], f32, tag="psA")
                for kc in range(4):
                    nc.tensor.matmul(
                        psA,
                        lhsT=wq_b[:, kc, ic * P : (ic + 1) * P],
                        rhs=wk_b[:, kc, :],
                        start=(kc == 0),
                        stop=(kc == 3),
                    )
                nc.vector.tensor_scalar_mul(A_sb[:, ic, :], psA, 1.0 / TEMPERATURE)

            # ---- WvT via TensorE transposes (f32 in, grouped 4-per-psum-bank)
            for jc in range(4):  # e_in slice -> WvT partition chunk
                pst = pstage.tile([P, 4 * P], f32, tag="pst")
                for ic in range(4):  # e_out chunk
                    nc.tensor.transpose(
                        pst[:, ic * P : (ic + 1) * P],
                        wv_f[:, ic, jc * P : (jc + 1) * P],
                        identity,
                    )
                nc.vector.tensor_copy(WvT[:, jc, :], pst)

            # ---- vT via TensorE transposes (f32 in, cast on copy-out)
            for ec in range(4):
                for g in range(2):  # two groups of 4 n-chunks
                    pst = pstage.tile([P, 4 * P], f32, tag="pst")
                    for j in range(4):
                        nk = g * 4 + j
                        nc.tensor.transpose(
                            pst[:, j * P : (j + 1) * P],
                            v_f32[:, nk, ec * P : (ec + 1) * P],
                            identity,
                        )
                    nc.vector.tensor_copy(
                        vT[:, ec, g * 512 : (g + 1) * 512], pst
                    )

            # ---- obs: cast -> bf16 scratch -> obsT via xbar DMA transpose
            # (writes + transposes dispatched from Sync; loads were on Scalar)
            scr_obs_r = scr_obs.rearrange("(c p) e -> p c e", p=P)
            for lc in range(8):
                of = obs_stage[lc]
                nc.vector.tensor_copy(obs_bf[:, lc * 4 : (lc + 1) * 4, :], of)
                nc.sync.dma_start(
                    scr_obs_r[:, lc * 4 : (lc + 1) * 4, :],
                    obs_bf[:, lc * 4 : (lc + 1) * 4, :],
                )
                for ec in range(4):
                    nc.sync.dma_start_transpose(
                        obsT[:, ec, lc * 512 : (lc + 1) * 512],
                        scr_obs[lc * 512 : (lc + 1) * 512, ec * P : (ec + 1) * P],
                    )

            # ---- vAT = (v @ A).T   [e2, n]
            for e2 in range(4):
                for nb in range(2):
                    psv = pstage.tile([P, 512], f32, tag="psv")
                    for e1 in range(4):
                        nc.tensor.matmul(
                            psv,
                            lhsT=A_sb[:, e1, e2 * P : (e2 + 1) * P],
                            rhs=vT[:, e1, nb * 512 : (nb + 1) * 512],
                            start=(e1 == 0),
                            stop=(e1 == 3),
                        )
                    nc.vector.tensor_copy(vAT[:, e2, nb * 512 : (nb + 1) * 512], psv)

            # ---- self score (token-major [n,1] per chunk) and w0 = exp(.)
            ps_sf = pstage.tile([P, 8], f32, tag="ps_sf")
            for ec in range(4):
                prod_ec = setup.tile([P, NLOC], bf16, tag="prod")
                nc.vector.tensor_mul(prod_ec, vAT[:, ec, :], vT[:, ec, :])
                for nk in range(8):
                    nc.tensor.matmul(
                        ps_sf[:, nk : nk + 1],
                        lhsT=prod_ec[:, nk * P : (nk + 1) * P],
                        rhs=ones_bf,
                        start=(ec == 0),
                        stop=(ec == 3),
                    )
            nc.scalar.activation(w0, ps_sf, AF.Exp)
            # w0 row-major broadcast [P, NLOC] via DRAM roundtrip (GpSimd)
            nc.gpsimd.dma_start(scr_w0.rearrange("o (a p) -> (o p) a", p=P), w0)
            nc.gpsimd.dma_start(w0_bc, bcast_ap(scr_w0[:]))

        # ---- main loop + fused epilogue
        with ExitStack() as mctx:
            ps_ut_pool = mctx.enter_context(
                tc.tile_pool(name="ps_ut", bufs=1, space="PSUM")
            )
            ps_s_pool = mctx.enter_context(
                tc.tile_pool(name="ps_s", bufs=2, space="PSUM")
            )
            ps_z_pool = mctx.enter_context(
                tc.tile_pool(name="ps_z", bufs=1, space="PSUM")
            )
            ps_y_pool = mctx.enter_context(
                tc.tile_pool(name="ps_y", bufs=1, space="PSUM")
            )
            for nb in range(2):
                nsl = slice(nb * 512, (nb + 1) * 512)
                ps_uT = ps_ut_pool.tile([P, 4, 512], f32, tag="uT")
                ps_z = ps_z_pool.tile([1, 512], f32, tag="z")
                for mc in range(32):
                    ps_s = ps_s_pool.tile([P, 512], f32, tag="s")
                    for ec in range(4):
                        nc.tensor.matmul(
                            ps_s,
                            lhsT=obsT[:, ec, mc * P : (mc + 1) * P],
                            rhs=vAT[:, ec, nsl],
                            start=(ec == 0),
                            stop=(ec == 3),
                        )
                    ex = expp.tile([P, 512], bf16, tag="ex")
                    nc.scalar.activation(ex, ps_s, AF.Exp)
                    nc.tensor.matmul(
                        ps_z, lhsT=ones_bf, rhs=ex, start=(mc == 0), stop=(mc == 31)
                    )
                    for es in range(4):
                        nc.tensor.matmul(
                            ps_uT[:, es, :],
                            lhsT=obs_bf[:, mc, es * P : (es + 1) * P],
                            rhs=ex,
                            start=(mc == 0),
                            stop=(mc == 31),
                        )
                # drain uT (+ fold in w0 * v term) and Z for this n-block
                for ec in range(4):
                    tmp = epi.tile([P, 512], bf16, tag="tmpw")
                    nc.vector.tensor_mul(tmp, vT[:, ec, nsl], w0_bc[:, nsl])
                    nc.vector.tensor_add(uT[:, ec, nsl], tmp, ps_uT[:, ec, :])
                zrow = epi.tile([1, 512], f32, tag="zrow")
                nc.vector.tensor_copy(zrow, ps_z)
                nc.gpsimd.dma_start(scr_z[:, nsl], zrow)
                nc.gpsimd.dma_start(
                    ztok[:, nb * 4 : (nb + 1) * 4],
                    scr_z[:, nsl].rearrange("o (a p) -> (o p) a", p=P),
                )
                c4 = slice(nb * 4, (nb + 1) * 4)
                nc.vector.tensor_add(ztok[:, c4], ztok[:, c4], w0[:, c4])
                nc.vector.reciprocal(recipZ[:, c4], ztok[:, c4])

                # epilogue for this n-block's 4 token chunks
                for nk in range(nb * 4, (nb + 1) * 4):
                    ps_y = ps_y_pool.tile([P, E], f32, tag="y")
                    for ec in range(4):
                        nc.tensor.matmul(
                            ps_y,
                            lhsT=uT[:, ec, nk * P : (nk + 1) * P],
                            rhs=WvT[:, ec, :],
                            start=(ec == 0),
                            stop=(ec == 3),
                        )
                    y2 = epi.tile([P, E], f32, tag="y2")
                    nc.vector.scalar_tensor_tensor(
                        y2,
                        in0=ps_y,
                        scalar=recipZ[:, nk : nk + 1],
                        in1=v_f32[:, nk, :],
                        op0=ALU.mult,
                        op1=ALU.add,
                    )
                    stats = epi.tile([P, 6], f32, tag="stats")
                    nc.vector.bn_stats(stats, y2)
                    mv = epi.tile([P, 2], f32, tag="mv")
                    nc.vector.bn_aggr(mv, stats)
                    std = epi.tile([P, 1], f32, tag="std")
                    nc.scalar.activation(std, mv[:, 1:2], AF.Sqrt, bias=eps_t)
                    rstd = epi.tile([P, 1], f32, tag="rstd")
                    nc.vector.reciprocal(rstd, std)
                    nc.vector.tensor_scalar(
                        y2,
                        in0=y2,
                        scalar1=mv[:, 0:1],
                        scalar2=rstd,
                        op0=ALU.subtract,
                        op1=ALU.mult,
                    )
                    nc.vector.tensor_mul(y2, y2, gamma_b)
                    nc.vector.tensor_add(y2, y2, beta_b)
                    nc.sync.dma_start(out_r[:, nk, :], y2)

    nc.compile()
    return nc


def _get_nc():
    global _CACHED_NC
    if _CACHED_NC is None:
        _CACHED_NC = _build()
    return _CACHED_NC


def _in_maps(v_code, obs_code, Wq, Wk, Wv, gamma, beta):
    def f(x):
        return np.ascontiguousarray(np.asarray(x), dtype=np.float32)

    shared = {
        "obs_code": f(obs_code),
        "Wq": f(Wq),
        "Wk": f(Wk),
        "Wv": f(Wv),
        "gamma": f(gamma),
        "beta": f(beta),
    }
    return [
        {"v_code": f(v_code[c * NLOC : (c + 1) * NLOC]), **shared}
        for c in range(CORES)
    ]


def run(trace=False, **inputs):
    from concourse.bass_utils import run_bass_kernel_spmd

    nc = _get_nc()
    res = run_bass_kernel_spmd(
        nc, _in_maps(**inputs), core_ids=list(range(CORES)), trace=trace
    )
    out = np.concatenate(
        [res.results[c]["out"] for c in range(CORES)], axis=0
    ).astype(np.float32)
    return out, res


def kernel(**inputs) -> np.ndarray:
    out, _ = run(trace=False, **inputs)
    return out
